# revision 14
# baseline (speedup 1.0000x reference)
"""AvgPool2d-as-Toeplitz kernel for Trainium2 (8 NeuronCores, SPMD).

Reference computes out = (enc_x * mask) @ W.T where W is the dense
Toeplitz matrix of conv2d with kernel ones(C,C,KH,KW)/(KH*KW) over the
flattened zero-padded input (C=16, KH=KW=2, stride 2, pad 1, H=W=32),
and mask zeroes the 1-pixel padding ring of each 34x34 channel image.

Structure exploited:
  W[(co,oi,oj), (ci,i,j)] = 0.25  iff  i in {2oi, 2oi+1} and j in {2oj, 2oj+1}
— independent of co, summed over every ci. With x viewed as
[B, C, 34, 34] and the mask folded in structurally (pooling windows
never read the masked border rows/columns):

  out[b, co, oi, oj] = 0.25 * sum_ci sum_window x[b, ci, i, j]
       over i in {2oi, 2oi+1} ∩ [1,32],  j in {2oj, 2oj+1} ∩ [1,32]

Per-core plan (4 batches per core, batch-parallel across 8 cores),
raw bacc with manual semaphores, optimized for latency:
  - Input DMA trimmed to image rows 1..32 (masked rows 0/33 never sent),
    split across the two HWDGE queues: ACT ring rows 1-17 (output half 1
    needs only these), SP ring rows 18-32.
  - DVE: column-pair adds + border-column copies + row-pair adds, half
    by half, so output half 1 flows while half 2's input is in flight.
  - PE: two single-pass fp32r matmuls (E exactly 0.25 => products are
    exact exponent shifts), one per output half, into separate PSUM
    banks.  E[(b,ci),(b2,co)] = 0.25*(b==b2) built by GPSIMD.
  - PSUM->SBUF copies: half 1 on GPSIMD (overlaps DVE's half-2 work),
    half 2 on DVE.
  - Output DMA split across both queues, issued as soon as each half is
    staged, with NO completion waits: the fixed ~7us NEFF teardown
    (semaphore-reset storm) overlaps the output DMA flight, and the
    runtime drains DGE queues before signalling completion.
"""

import sys

import numpy as np

if "/opt/trn_rl_repo" not in sys.path:
    sys.path.insert(0, "/opt/trn_rl_repo")

B, C = 32, 16
HP = WP = 34
OH = OW = 17
IMG = HP * WP             # 1156
IN_DIM = C * IMG          # 18496
OUT_DIM = C * OH * OW     # 4624
N_CORES = 8
B_SH = B // N_CORES       # 4 batches per core
P = B_SH * C              # 64 partitions in use

OI1 = 9                   # output rows in half 1 (oi 0..8 <- image rows 0..17)
N1 = OI1 * OW             # 153
N2 = (OH - OI1) * OW      # 136
GAP = 16                  # sacrificial staging gap: the ACT-engine PSUM copy
                          # writes past its slice in coarse granularity
OFF2 = N1 + 1 + GAP       # staging offset of output half 2

_PROGRAM = None


def _build_program():
    import concourse.bacc as bacc
    import concourse.mybir as mybir

    f32 = mybir.dt.float32
    f32r = mybir.dt.float32r
    add = mybir.AluOpType.add
    nc = bacc.Bacc()

    x = nc.declare_dram_parameter("x", [B_SH, IN_DIM], f32, isOutput=False)
    out = nc.declare_dram_parameter("out", [B_SH, OUT_DIM], f32, isOutput=True)
    xv = x[:, :].rearrange("b (c f) -> (b c) f", c=C)   # [64, 1156]
    ov = out[:, :].rearrange("b (co s) -> (b co) s", co=C)

    with (
        nc.sbuf_tensor([P, IMG], f32) as xt,
        nc.sbuf_tensor([P, P], f32) as et,
        nc.sbuf_tensor([P, P], f32r) as etr,
        nc.sbuf_tensor([P, HP * OW], f32) as at,
        nc.sbuf_tensor([P, N1 + 1 + N2], f32r) as a2t,
        nc.sbuf_tensor([P, OFF2 + N2], f32) as ot,
        nc.sbuf_tensor([P, 8], f32) as scr,
        nc.psum_tensor([P, N1 + 1], f32) as pt0,
        nc.psum_tensor([P, N2], f32) as pt1,
        nc.semaphore("s_a") as s_a,
        nc.semaphore("s_b") as s_b,
        nc.semaphore("s_gps") as s_gps,
        nc.semaphore("s_dve") as s_dve,
        nc.semaphore("s_pe") as s_pe,
        nc.semaphore("s_cp1") as s_cp1,
        nc.semaphore("s_cp2") as s_cp2,
        nc.semaphore("s_out") as s_out,
        nc.Block() as block,
    ):
        x3 = xt[:].rearrange("p (i j) -> p i j", i=HP)
        a3 = at[:].rearrange("p (i oj) -> p i oj", i=HP)
        a23a = a2t[:, 0:N1].rearrange("p (oi oj) -> p oi oj", oi=OI1)
        a23b = a2t[:, N1 + 1:N1 + 1 + N2].rearrange(
            "p (oi oj) -> p oi oj", oi=OH - OI1
        )
        e3 = et[:].rearrange("p (qb qc) -> p qb qc", qb=B_SH)

        RS = 18  # image-row split: ACT ring rows 1..17, SP ring rows 18..32

        @block.scalar
        def _(scalar):
            # rows 1-17 (oi 0..8 window) on the ACT ring
            scalar.dma_start(
                xt[:, WP:RS * WP], xv[:, WP:RS * WP]
            ).then_inc(s_a, 16)
            # dummy activation: forces the ACT table load to be placed
            # here, after the input DMA's descriptors are in the ring
            nc.scalar.copy(scr[:, 0:2], scr[:, 2:4])
            # stage + ship output half 1; no completion wait: the NEFF
            # teardown overlaps the output flight and the runtime drains
            # the queues before completion
            scalar.wait_ge(s_pe, 1)
            nc.scalar.copy(ot[:, 0:N1 + 1], pt0[:]).then_inc(s_cp1, 1)
            scalar.dma_start(
                ov[:, 0:N1], ot[:, 0:N1], single_packet=True
            ).then_inc(s_out, 16)._wait_ge(s_cp1, 1)

        @block.sync
        def _(sync):
            # rows 18-32 on the SP ring (row 33 is masked, never sent)
            sync.dma_start(
                xt[:, RS * WP:IMG - WP], xv[:, RS * WP:IMG - WP]
            ).then_inc(s_b, 16)
            # output half 2; no completion wait
            sync.wait_ge(s_cp2, 1)
            sync.dma_start(
                ov[:, N1:OH * OW], ot[:, OFF2:OFF2 + N2], single_packet=True
            ).then_inc(s_out, 16)

        @block.gpsimd
        def _(gpsimd):
            # masked image rows 0 and 33 of the column-pooled tile -> 0,
            # so the row-pair stage needs no border special-casing
            gpsimd.memset(a3[:, 0:HP:HP - 1, :], 0.0).then_inc(s_gps, 1)
            # E[p,(qb,qc)] = 0.25 iff 0 <= p - 16*qb <= 15
            gpsimd.memset(et[:], 0.25).then_inc(s_gps, 1)
            gpsimd.wait_ge(s_gps, 2)
            nc.gpsimd.affine_select(
                e3, e3, [[-C, B_SH], [0, C]], mybir.AluOpType.is_ge, 0.0,
                base=0, channel_multiplier=1,
            ).then_inc(s_gps, 1)
            gpsimd.wait_ge(s_gps, 3)
            nc.gpsimd.affine_select(
                e3, e3, [[C, B_SH], [0, C]], mybir.AluOpType.is_ge, 0.0,
                base=C - 1, channel_multiplier=-1,
            ).then_inc(s_gps, 1)
            # half-2 column ops in parallel with DVE's half-1 work
            gpsimd.wait_ge(s_b, 16)
            nc.gpsimd.tensor_tensor(
                a3[:, RS:HP - 1, 1:16],
                x3[:, RS:HP - 1, 2:32:2], x3[:, RS:HP - 1, 3:33:2], add,
            ).then_inc(s_gps, 1)
            nc.gpsimd.tensor_copy(
                a3[:, RS:HP - 1, 0:17:16], x3[:, RS:HP - 1, 1:33:31]
            ).then_inc(s_gps, 1)._wait_ge(s_gps, 5)

        @block.vector
        def _(vector):
            def ctt(r0, r1):
                return nc.vector.tensor_tensor(
                    a3[:, r0:r1, 1:16],
                    x3[:, r0:r1, 2:32:2], x3[:, r0:r1, 3:33:2], add,
                )

            def cb(r0, r1):
                return nc.vector.tensor_copy(
                    a3[:, r0:r1, 0:17:16], x3[:, r0:r1, 1:33:31]
                )

            # round E to fp32r for the single-pass matmuls, and zero the
            # even-width pad column of a2 half 1 (fp32r needs even N)
            vector.wait_ge(s_gps, 4)
            nc.vector.tensor_copy(etr[:], et[:]).then_inc(s_dve, 1)
            nc.vector.tensor_copy(
                a2t[:, N1:N1 + 1], a3[:, 0, 0:1]
            ).then_inc(s_dve, 1)                          # s_dve = 2
            vector.wait_ge(s_a, 16)
            ctt(1, RS).then_inc(s_dve, 1)                 # rows 1-17
            cb(1, RS).then_inc(s_dve, 1)
            # a2 rows oi 0..8 from a rows 0..17 (row 0 pre-zeroed by GPS)
            nc.vector.tensor_tensor(
                a23a[:], a3[:, 0:RS:2, :], a3[:, 1:RS:2, :], add,
            ).then_inc(s_dve, 1)._wait_ge(s_dve, 4)       # s_dve = 5
            # a2 rows oi 9..16 from a rows 18..33 (GPSIMD column ops,
            # row 33 pre-zeroed)
            vector.wait_ge(s_gps, 6)
            nc.vector.tensor_tensor(
                a23b[:], a3[:, RS:HP:2, :], a3[:, RS + 1:HP:2, :],
                add,
            ).then_inc(s_dve, 1)                          # s_dve = 6
            # stage output half 2
            vector.wait_ge(s_pe, 2)
            nc.vector.tensor_copy(
                ot[:, OFF2:OFF2 + N2], pt1[:]
            ).then_inc(s_cp2, 1)

        @block.tensor
        def _(tensor):
            # single-pass fp32r matmuls: E is exactly 0.25, so products are
            # exact; accumulation stays fp32 in PSUM
            tensor.wait_ge(s_dve, 5)
            nc.tensor.matmul(
                pt0[:], etr[:], a2t[:, 0:N1 + 1],
                start=True, stop=True,
            ).then_inc(s_pe, 1)
            tensor.wait_ge(s_dve, 6)
            nc.tensor.matmul(
                pt1[:], etr[:], a2t[:, N1 + 1:N1 + 1 + N2],
                start=True, stop=True,
            ).then_inc(s_pe, 1)

    nc.compile()
    return nc


def _get_program():
    global _PROGRAM
    if _PROGRAM is None:
        _PROGRAM = _build_program()
    return _PROGRAM


def _run(enc_x: np.ndarray, mask: np.ndarray = None, **spmd_kwargs):
    from concourse.bass_utils import run_bass_kernel_spmd

    nc = _get_program()
    in_maps = []
    for i in range(N_CORES):
        sl = slice(i * B_SH, (i + 1) * B_SH)
        in_maps.append({"x": np.ascontiguousarray(enc_x[sl], dtype=np.float32)})
    res = run_bass_kernel_spmd(nc, in_maps, list(range(N_CORES)), **spmd_kwargs)
    out = np.concatenate([res.results[i]["out"] for i in range(N_CORES)], axis=0)
    return out, res


def kernel(enc_x, weight=None, mask=None, **_unused):
    enc_x = np.asarray(enc_x, dtype=np.float32)
    assert enc_x.shape == (B, IN_DIM), enc_x.shape
    out, _ = _run(enc_x)
    return out


# revision 15
# speedup vs baseline: 1.0093x; 1.0093x over previous
"""AvgPool2d-as-Toeplitz kernel for Trainium2 (8 NeuronCores, SPMD).

Reference computes out = (enc_x * mask) @ W.T where W is the dense
Toeplitz matrix of conv2d with kernel ones(C,C,KH,KW)/(KH*KW) over the
flattened zero-padded input (C=16, KH=KW=2, stride 2, pad 1, H=W=32),
and mask zeroes the 1-pixel padding ring of each 34x34 channel image.

Structure exploited:
  W[(co,oi,oj), (ci,i,j)] = 0.25  iff  i in {2oi, 2oi+1} and j in {2oj, 2oj+1}
— independent of co, summed over every ci. With x viewed as
[B, C, 34, 34] and the mask folded in structurally (pooling windows
never read the masked border rows/columns):

  out[b, co, oi, oj] = 0.25 * sum_ci sum_window x[b, ci, i, j]
       over i in {2oi, 2oi+1} ∩ [1,32],  j in {2oj, 2oj+1} ∩ [1,32]

Per-core plan (4 batches per core, batch-parallel across 8 cores),
raw bacc with manual semaphores, optimized for latency:
  - Input DMA trimmed to image rows 1..32 and split into four chunks at
    pooling-pair boundaries, two per HWDGE queue (ACT: rows 1-9, 10-17;
    SP: rows 18-25, 26-32), so vector work starts as soon as the first
    chunk lands while the rest is in flight.
  - DVE: A-half column-pair adds + border copies, all row-pair adds,
    fp32->fp32r rounding casts, and both PSUM->SBUF copies.  GPSIMD
    covers the B-half column ops in parallel (after building
    E[(b,ci),(b2,co)] = 0.25*(b==b2)).
  - PE: two single-pass fp32r matmuls (E is exactly 0.25 => products
    are exact exponent shifts), one per output half, into separate PSUM
    banks; fp32r needs an even moving width, so half 1 carries one
    zeroed pad column.
  - Output DMA split across both queues, issued as soon as each half is
    staged, with NO completion waits: the fixed ~7us NEFF teardown
    (semaphore-reset storm) overlaps the output DMA flight, and the
    runtime drains DGE queues before completion.
  - Engines execute with relaxed ordering, so readers that follow
    same-engine writers carry attached semaphore waits.
"""

import sys

import numpy as np

if "/opt/trn_rl_repo" not in sys.path:
    sys.path.insert(0, "/opt/trn_rl_repo")

B, C = 32, 16
HP = WP = 34
OH = OW = 17
IMG = HP * WP             # 1156
IN_DIM = C * IMG          # 18496
OUT_DIM = C * OH * OW     # 4624
N_CORES = 8
B_SH = B // N_CORES       # 4 batches per core
P = B_SH * C              # 64 partitions in use

OI1 = 9                   # output rows in half 1 (oi 0..8 <- image rows 0..17)
N1 = OI1 * OW             # 153
N2 = (OH - OI1) * OW      # 136
GAP = 16                  # sacrificial staging gap between output halves
OFF2 = N1 + 1 + GAP       # staging offset of output half 2

# input chunk boundaries (image rows, even = pooling-pair aligned):
# A1 = rows 1..9, A2 = 10..17, B1 = 18..25, B2 = 26..32
R1, R2, R3 = 10, 18, 26

_PROGRAM = None


def _build_program():
    import concourse.bacc as bacc
    import concourse.mybir as mybir

    f32 = mybir.dt.float32
    f32r = mybir.dt.float32r
    add = mybir.AluOpType.add
    nc = bacc.Bacc()

    x = nc.declare_dram_parameter("x", [B_SH, IN_DIM], f32, isOutput=False)
    out = nc.declare_dram_parameter("out", [B_SH, OUT_DIM], f32, isOutput=True)
    xv = x[:, :].rearrange("b (c f) -> (b c) f", c=C)   # [64, 1156]
    ov = out[:, :].rearrange("b (co s) -> (b co) s", co=C)

    with (
        nc.sbuf_tensor([P, IMG], f32) as xt,
        nc.sbuf_tensor([P, P], f32) as et,
        nc.sbuf_tensor([P, P], f32r) as etr,
        nc.sbuf_tensor([P, HP * OW], f32) as at,
        nc.sbuf_tensor([P, N1 + 1 + N2], f32r) as a2t,
        nc.sbuf_tensor([P, OFF2 + N2], f32) as ot,
        nc.psum_tensor([P, N1 + 1], f32) as pt0,
        nc.psum_tensor([P, N2], f32) as pt1,
        nc.semaphore("s_a1") as s_a1,
        nc.semaphore("s_a2") as s_a2,
        nc.semaphore("s_b1") as s_b1,
        nc.semaphore("s_b2") as s_b2,
        nc.semaphore("s_gps") as s_gps,
        nc.semaphore("s_gc") as s_gc,
        nc.semaphore("s_dve") as s_dve,
        nc.semaphore("s_pe") as s_pe,
        nc.semaphore("s_cp1") as s_cp1,
        nc.semaphore("s_cp2") as s_cp2,
        nc.semaphore("s_out") as s_out,
        nc.Block() as block,
    ):
        x3 = xt[:].rearrange("p (i j) -> p i j", i=HP)
        a3 = at[:].rearrange("p (i oj) -> p i oj", i=HP)
        a23a = a2t[:, 0:N1].rearrange("p (oi oj) -> p oi oj", oi=OI1)
        a23b = a2t[:, N1 + 1:N1 + 1 + N2].rearrange(
            "p (oi oj) -> p oi oj", oi=OH - OI1
        )
        e3 = et[:].rearrange("p (qb qc) -> p qb qc", qb=B_SH)

        def ctt(eng, r0, r1):
            # column-pair add for interior output columns oj 1..15
            return eng.tensor_tensor(
                a3[:, r0:r1, 1:16],
                x3[:, r0:r1, 2:32:2], x3[:, r0:r1, 3:33:2], add,
            )

        def cb(eng, r0, r1):
            # border output columns oj 0 / 16 <- image columns 1 / 32
            return eng.tensor_copy(
                a3[:, r0:r1, 0:17:16], x3[:, r0:r1, 1:33:31]
            )

        @block.scalar
        def _(scalar):
            scalar.dma_start(
                xt[:, WP:R1 * WP], xv[:, WP:R1 * WP]
            ).then_inc(s_a1, 16)
            scalar.dma_start(
                xt[:, R1 * WP:R2 * WP], xv[:, R1 * WP:R2 * WP]
            ).then_inc(s_a2, 16)
            # ship output half 1 once staged; no completion wait: the NEFF
            # teardown overlaps the output flight and the runtime drains
            # the DGE queues before completion
            scalar.wait_ge(s_cp1, 1)
            scalar.dma_start(
                ov[:, 0:N1], ot[:, 0:N1], single_packet=True
            ).then_inc(s_out, 16)

        @block.sync
        def _(sync):
            sync.dma_start(
                xt[:, R2 * WP:R3 * WP], xv[:, R2 * WP:R3 * WP]
            ).then_inc(s_b1, 16)
            sync.dma_start(
                xt[:, R3 * WP:IMG - WP], xv[:, R3 * WP:IMG - WP]
            ).then_inc(s_b2, 16)
            sync.wait_ge(s_cp2, 1)
            sync.dma_start(
                ov[:, N1:OH * OW], ot[:, OFF2:OFF2 + N2], single_packet=True
            ).then_inc(s_out, 16)

        @block.gpsimd
        def _(gpsimd):
            # masked image rows 0 and 33 of the column-pooled tile -> 0,
            # so the row-pair stage needs no border special-casing
            gpsimd.memset(a3[:, 0:HP:HP - 1, :], 0.0).then_inc(s_gps, 1)
            # E[p,(qb,qc)] = 0.25 iff 0 <= p - 16*qb <= 15
            gpsimd.memset(et[:], 0.25).then_inc(s_gps, 1)
            gpsimd.wait_ge(s_gps, 2)
            nc.gpsimd.affine_select(
                e3, e3, [[-C, B_SH], [0, C]], mybir.AluOpType.is_ge, 0.0,
                base=0, channel_multiplier=1,
            ).then_inc(s_gps, 1)
            gpsimd.wait_ge(s_gps, 3)
            nc.gpsimd.affine_select(
                e3, e3, [[C, B_SH], [0, C]], mybir.AluOpType.is_ge, 0.0,
                base=C - 1, channel_multiplier=-1,
            ).then_inc(s_gps, 1)
            # B-half column ops in parallel with DVE's A-half work
            gpsimd.wait_ge(s_b1, 16)
            ctt(nc.gpsimd, R2, R3).then_inc(s_gc, 1)
            cb(nc.gpsimd, R2, R3).then_inc(s_gc, 1)
            gpsimd.wait_ge(s_b2, 16)
            ctt(nc.gpsimd, R3, HP - 1).then_inc(s_gc, 1)
            cb(nc.gpsimd, R3, HP - 1).then_inc(s_gc, 1)

        @block.vector
        def _(vector):
            # round E to fp32r for the single-pass matmuls, and zero the
            # even-width pad column of a2 half 1 (fp32r needs even N)
            vector.wait_ge(s_gps, 4)
            nc.vector.tensor_copy(etr[:], et[:]).then_inc(s_dve, 1)
            nc.vector.tensor_copy(
                a2t[:, N1:N1 + 1], a3[:, 0, 0:1]
            ).then_inc(s_dve, 1)                          # s_dve = 2
            vector.wait_ge(s_a1, 16)
            ctt(nc.vector, 1, R1).then_inc(s_dve, 1)      # rows 1-9
            cb(nc.vector, 1, R1).then_inc(s_dve, 1)
            # oi 0..4 from a rows 0..9 (row 0 pre-zeroed by GPS)
            nc.vector.tensor_tensor(
                a23a[:, 0:5, :], a3[:, 0:R1:2, :], a3[:, 1:R1:2, :], add,
            ).then_inc(s_dve, 1)._wait_ge(s_dve, 4)       # s_dve = 5
            vector.wait_ge(s_a2, 16)
            ctt(nc.vector, R1, R2).then_inc(s_dve, 1)     # rows 10-17
            cb(nc.vector, R1, R2).then_inc(s_dve, 1)
            # oi 5..8 from a rows 10..17
            nc.vector.tensor_tensor(
                a23a[:, 5:OI1, :], a3[:, R1:R2:2, :], a3[:, R1 + 1:R2:2, :],
                add,
            ).then_inc(s_dve, 1)._wait_ge(s_dve, 7)       # s_dve = 8
            # oi 9..12 from a rows 18..25 (GPSIMD column ops)
            vector.wait_ge(s_gc, 2)
            nc.vector.tensor_tensor(
                a23b[:, 0:4, :], a3[:, R2:R3:2, :], a3[:, R2 + 1:R3:2, :],
                add,
            ).then_inc(s_dve, 1)                          # s_dve = 9
            # oi 13..16 from a rows 26..33 (row 33 pre-zeroed)
            vector.wait_ge(s_gc, 4)
            nc.vector.tensor_tensor(
                a23b[:, 4:OH - OI1, :], a3[:, R3:HP:2, :],
                a3[:, R3 + 1:HP:2, :], add,
            ).then_inc(s_dve, 1)                          # s_dve = 10
            # stage output half 1 (overlaps matmul 2), then half 2
            vector.wait_ge(s_pe, 1)
            nc.vector.tensor_copy(ot[:, 0:N1], pt0[:, 0:N1]).then_inc(
                s_cp1, 1
            )
            vector.wait_ge(s_pe, 2)
            nc.vector.tensor_copy(
                ot[:, OFF2:OFF2 + N2], pt1[:]
            ).then_inc(s_cp2, 1)

        @block.tensor
        def _(tensor):
            # single-pass fp32r matmuls: E is exactly 0.25, so products are
            # exact; accumulation stays fp32 in PSUM
            tensor.wait_ge(s_dve, 8)
            nc.tensor.matmul(
                pt0[:], etr[:], a2t[:, 0:N1 + 1],
                start=True, stop=True,
            ).then_inc(s_pe, 1)
            tensor.wait_ge(s_dve, 10)
            nc.tensor.matmul(
                pt1[:], etr[:], a2t[:, N1 + 1:N1 + 1 + N2],
                start=True, stop=True,
            ).then_inc(s_pe, 1)

    nc.compile()
    return nc


def _get_program():
    global _PROGRAM
    if _PROGRAM is None:
        _PROGRAM = _build_program()
    return _PROGRAM


def _run(enc_x: np.ndarray, mask: np.ndarray = None, **spmd_kwargs):
    from concourse.bass_utils import run_bass_kernel_spmd

    nc = _get_program()
    in_maps = []
    for i in range(N_CORES):
        sl = slice(i * B_SH, (i + 1) * B_SH)
        in_maps.append({"x": np.ascontiguousarray(enc_x[sl], dtype=np.float32)})
    res = run_bass_kernel_spmd(nc, in_maps, list(range(N_CORES)), **spmd_kwargs)
    out = np.concatenate([res.results[i]["out"] for i in range(N_CORES)], axis=0)
    return out, res


def kernel(enc_x, weight=None, mask=None, **_unused):
    enc_x = np.asarray(enc_x, dtype=np.float32)
    assert enc_x.shape == (B, IN_DIM), enc_x.shape
    out, _ = _run(enc_x)
    return out


# revision 16
# speedup vs baseline: 1.1972x; 1.1862x over previous
"""AvgPool2d-as-Toeplitz kernel for Trainium2 (8 NeuronCores, SPMD).

Reference computes out = (enc_x * mask) @ W.T where W is the dense
Toeplitz matrix of conv2d with kernel ones(C,C,KH,KW)/(KH*KW) over the
flattened zero-padded input (C=16, KH=KW=2, stride 2, pad 1, H=W=32),
and mask zeroes the 1-pixel padding ring of each 34x34 channel image.

Structure exploited:
  W[(co,oi,oj), (ci,i,j)] = 0.25  iff  i in {2oi, 2oi+1} and j in {2oj, 2oj+1}
— independent of co, summed over every ci. With x viewed as
[B, C, 34, 34] and the mask folded in structurally (pooling windows
never read the masked border rows/columns):

  out[b, co, oi, oj] = 0.25 * sum_ci sum_window x[b, ci, i, j]
       over i in {2oi, 2oi+1} ∩ [1,32],  j in {2oj, 2oj+1} ∩ [1,32]

Per-core plan (4 batches per core, batch-parallel across 8 cores),
raw bacc with manual semaphores, optimized for latency:
  - Input DMA trimmed to image rows 1..32 and split into four chunks at
    pooling-pair boundaries, two per HWDGE queue (ACT: rows 1-9, 10-17;
    SP: rows 18-25, 26-32), so vector work starts as soon as the first
    chunk lands while the rest is in flight.
  - DVE: A-half column-pair adds + border copies, all row-pair adds,
    fp32->fp32r rounding casts, and both PSUM->SBUF copies.  GPSIMD
    covers the B-half column ops in parallel (after building
    E[(b,ci),(b2,co)] = 0.25*(b==b2)).
  - PE: two single-pass fp32r matmuls (E is exactly 0.25 => products
    are exact exponent shifts), one per output half, into separate PSUM
    banks; fp32r needs an even moving width, so half 1 carries one
    zeroed pad column.
  - Output DMA split across both queues, issued as soon as each half is
    staged, with NO completion waits: the fixed ~7us NEFF teardown
    (semaphore-reset storm) overlaps the output DMA flight, and the
    runtime drains DGE queues before completion.
  - Engines execute with relaxed ordering, so readers that follow
    same-engine writers carry attached semaphore waits.
"""

import sys

import numpy as np

if "/opt/trn_rl_repo" not in sys.path:
    sys.path.insert(0, "/opt/trn_rl_repo")

B, C = 32, 16
HP = WP = 34
OH = OW = 17
IMG = HP * WP             # 1156
IN_DIM = C * IMG          # 18496
OUT_DIM = C * OH * OW     # 4624
N_CORES = 8
B_SH = B // N_CORES       # 4 batches per core
P = B_SH * C              # 64 partitions in use

OI1 = 9                   # output rows in half 1 (oi 0..8 <- image rows 0..17)
N1 = OI1 * OW             # 153
N2 = (OH - OI1) * OW      # 136
GAP = 16                  # sacrificial staging gap between output halves
OFF2 = N1 + 1 + GAP       # staging offset of output half 2

RS = 18                   # image-row split: SP ring rows 1..17, ACT rows 18..32

_PROGRAM = None


def _build_program():
    import concourse.bacc as bacc
    import concourse.mybir as mybir

    f32 = mybir.dt.float32
    f32r = mybir.dt.float32r
    add = mybir.AluOpType.add
    nc = bacc.Bacc()

    x = nc.declare_dram_parameter("x", [B_SH, IN_DIM], f32, isOutput=False)
    out = nc.declare_dram_parameter("out", [B_SH, OUT_DIM], f32, isOutput=True)
    xv = x[:, :].rearrange("b (c f) -> (b c) f", c=C)   # [64, 1156]
    ov = out[:, :].rearrange("b (co s) -> (b co) s", co=C)

    with (
        nc.sbuf_tensor([P, IMG], f32) as xt,
        nc.sbuf_tensor([P, P], f32) as et,
        nc.sbuf_tensor([P, P], f32r) as etr,
        nc.sbuf_tensor([P, HP * OW], f32) as at,
        nc.sbuf_tensor([P, N1 + 1 + N2], f32r) as a2t,
        nc.sbuf_tensor([P, OFF2 + N2], f32) as ot,
        nc.psum_tensor([P, N1 + 1], f32) as pt0,
        nc.psum_tensor([P, N2], f32) as pt1,
        nc.semaphore("s_a1") as s_a1,
        nc.semaphore("s_b1") as s_b1,
        nc.semaphore("s_gps") as s_gps,
        nc.semaphore("s_dve") as s_dve,
        nc.semaphore("s_pe") as s_pe,
        nc.semaphore("s_cp1") as s_cp1,
        nc.semaphore("s_cp2") as s_cp2,
        nc.semaphore("s_out") as s_out,
        nc.Block() as block,
    ):
        x3 = xt[:].rearrange("p (i j) -> p i j", i=HP)
        a3 = at[:].rearrange("p (i oj) -> p i oj", i=HP)
        a23a = a2t[:, 0:N1].rearrange("p (oi oj) -> p oi oj", oi=OI1)
        a23b = a2t[:, N1 + 1:N1 + 1 + N2].rearrange(
            "p (oi oj) -> p oi oj", oi=OH - OI1
        )
        e3 = et[:].rearrange("p (qb qc) -> p qb qc", qb=B_SH)

        def ctt(eng, r0, r1):
            # column-pair add for interior output columns oj 1..15
            return eng.tensor_tensor(
                a3[:, r0:r1, 1:16],
                x3[:, r0:r1, 2:32:2], x3[:, r0:r1, 3:33:2], add,
            )

        def cb(eng, r0, r1):
            # border output columns oj 0 / 16 <- image columns 1 / 32
            return eng.tensor_copy(
                a3[:, r0:r1, 0:17:16], x3[:, r0:r1, 1:33:31]
            )

        @block.sync
        def _(sync):
            # rows 1..17 (output half 1) on the SP ring -- it consistently
            # delivers earlier than the ACT ring
            sync.dma_start(
                xt[:, WP:RS * WP], xv[:, WP:RS * WP]
            ).then_inc(s_a1, 16)
            # ship output half 2 once staged; no completion wait: the NEFF
            # teardown overlaps the output flight and the runtime drains
            # the DGE queues before completion
            sync.wait_ge(s_cp2, 1)
            sync.dma_start(
                ov[:, N1:OH * OW], ot[:, OFF2:OFF2 + N2], single_packet=True
            ).then_inc(s_out, 16)

        @block.scalar
        def _(scalar):
            # rows 18..32 (output half 2) on the ACT ring
            scalar.dma_start(
                xt[:, RS * WP:IMG - WP], xv[:, RS * WP:IMG - WP]
            ).then_inc(s_b1, 16)
            scalar.wait_ge(s_cp1, 1)
            scalar.dma_start(
                ov[:, 0:N1], ot[:, 0:N1], single_packet=True
            ).then_inc(s_out, 16)

        @block.gpsimd
        def _(gpsimd):
            # masked image rows 0 and 33 of the column-pooled tile -> 0,
            # so the row-pair stage needs no border special-casing
            gpsimd.memset(a3[:, 0:HP:HP - 1, :], 0.0).then_inc(s_gps, 1)
            # E[p,(qb,qc)] = 0.25 iff 0 <= p - 16*qb <= 15
            gpsimd.memset(et[:], 0.25).then_inc(s_gps, 1)
            gpsimd.wait_ge(s_gps, 2)
            nc.gpsimd.affine_select(
                e3, e3, [[-C, B_SH], [0, C]], mybir.AluOpType.is_ge, 0.0,
                base=0, channel_multiplier=1,
            ).then_inc(s_gps, 1)
            gpsimd.wait_ge(s_gps, 3)
            nc.gpsimd.affine_select(
                e3, e3, [[C, B_SH], [0, C]], mybir.AluOpType.is_ge, 0.0,
                base=C - 1, channel_multiplier=-1,
            ).then_inc(s_gps, 1)

        @block.vector
        def _(vector):
            # round E to fp32r for the single-pass matmuls, and zero the
            # even-width pad column of a2 half 1 (fp32r needs even N)
            vector.wait_ge(s_gps, 4)
            nc.vector.tensor_copy(etr[:], et[:]).then_inc(s_dve, 1)
            nc.vector.tensor_copy(
                a2t[:, N1:N1 + 1], a3[:, 0, 0:1]
            ).then_inc(s_dve, 1)                          # s_dve = 2
            vector.wait_ge(s_a1, 16)
            ctt(nc.vector, 1, RS).then_inc(s_dve, 1)      # rows 1-17
            cb(nc.vector, 1, RS).then_inc(s_dve, 1)
            # oi 0..8 from a rows 0..17 (row 0 pre-zeroed by GPS)
            nc.vector.tensor_tensor(
                a23a[:], a3[:, 0:RS:2, :], a3[:, 1:RS:2, :], add,
            ).then_inc(s_dve, 1)._wait_ge(s_dve, 4)       # s_dve = 5
            vector.wait_ge(s_b1, 16)
            ctt(nc.vector, RS, HP - 1).then_inc(s_dve, 1)  # rows 18-32
            cb(nc.vector, RS, HP - 1).then_inc(s_dve, 1)
            # oi 9..16 from a rows 18..33 (row 33 pre-zeroed)
            nc.vector.tensor_tensor(
                a23b[:], a3[:, RS:HP:2, :], a3[:, RS + 1:HP:2, :], add,
            ).then_inc(s_dve, 1)._wait_ge(s_dve, 7)       # s_dve = 8
            # stage output half 1 (overlaps matmul 2), then half 2
            vector.wait_ge(s_pe, 1)
            nc.vector.tensor_copy(ot[:, 0:N1], pt0[:, 0:N1]).then_inc(
                s_cp1, 1
            )
            vector.wait_ge(s_pe, 2)
            nc.vector.tensor_copy(
                ot[:, OFF2:OFF2 + N2], pt1[:]
            ).then_inc(s_cp2, 1)

        @block.tensor
        def _(tensor):
            # single-pass fp32r matmuls: E is exactly 0.25, so products are
            # exact; accumulation stays fp32 in PSUM
            tensor.wait_ge(s_dve, 5)
            nc.tensor.matmul(
                pt0[:], etr[:], a2t[:, 0:N1 + 1],
                start=True, stop=True,
            ).then_inc(s_pe, 1)
            tensor.wait_ge(s_dve, 8)
            nc.tensor.matmul(
                pt1[:], etr[:], a2t[:, N1 + 1:N1 + 1 + N2],
                start=True, stop=True,
            ).then_inc(s_pe, 1)

    nc.compile()
    return nc


def _get_program():
    global _PROGRAM
    if _PROGRAM is None:
        _PROGRAM = _build_program()
    return _PROGRAM


def _run(enc_x: np.ndarray, mask: np.ndarray = None, **spmd_kwargs):
    from concourse.bass_utils import run_bass_kernel_spmd

    nc = _get_program()
    in_maps = []
    for i in range(N_CORES):
        sl = slice(i * B_SH, (i + 1) * B_SH)
        in_maps.append({"x": np.ascontiguousarray(enc_x[sl], dtype=np.float32)})
    res = run_bass_kernel_spmd(nc, in_maps, list(range(N_CORES)), **spmd_kwargs)
    out = np.concatenate([res.results[i]["out"] for i in range(N_CORES)], axis=0)
    return out, res


def kernel(enc_x, weight=None, mask=None, **_unused):
    enc_x = np.asarray(enc_x, dtype=np.float32)
    assert enc_x.shape == (B, IN_DIM), enc_x.shape
    out, _ = _run(enc_x)
    return out
